# revision 25
# baseline (speedup 1.0000x reference)
"""DiGCN (2-layer DIGCNConv + BatchNorm + per-graph MHA + LayerNorm) on 8
Trainium2 NeuronCores.

Strategy (sharding_hint: data-parallel over graphs):
  - 64 graphs of 512 nodes -> core c owns graphs 8c..8c+7 (4096 dst nodes).
  - Edges are partitioned by dst core and CSR-sorted by dst on the host;
    per 128-dst tile they are padded to a uniform chunk grid.  The
    segment-sum  conv[d] += attr_e * hW[src_e]  is computed as a chain of
    PE matmuls against host-built "indicator" blocks Ind[e, d] =
    attr_e * (dst_e == d), which also folds in edge_attr.
  - conv = A @ (h @ W) is commuted to (A @ h) @ W so layer 0 gathers rows
    of x directly (fp32, no table build).  Layer 1 gathers from an
    AllGather'ed bf16 copy of h1.
  - Attention is computed in transposed layouts (scoresT = K Q^T) so no
    512x512 transposes are needed; softmax denominators come from
    ones-vector matmuls, and normalization uses gpsimd partition_broadcast.
"""

import os
import sys

for _p in ("/opt/trn_rl_repo", "/root/.axon_site/_ro/trn_rl_repo"):
    if os.path.isdir(_p) and _p not in sys.path:
        sys.path.insert(0, _p)

import numpy as np

import concourse.bacc as bacc
import concourse.bass as bass
import concourse.tile as tile
from concourse import mybir
from concourse.bass_utils import run_bass_kernel_spmd
from concourse.library_config import mlp as _mlp_lib
from concourse.masks import make_identity

F32 = mybir.dt.float32
BF16 = mybir.dt.bfloat16
I16 = mybir.dt.int16
NP_BF16 = mybir.dt.np(BF16)
AF = mybir.ActivationFunctionType
OP = mybir.AluOpType

N = 32768
E = 524288
D = 256          # NFEAT == NHID
HEADS = 4
DH = 64
NCORES = 8
NL = N // NCORES         # 4096 nodes per core
G = 8                    # graphs per core
S = 512                  # nodes per graph
TILES = NL // 128        # 32 dst tiles per core
EPS = 1e-5

# ---------------------------------------------------------------------------
# Host-side edge preprocessing (index/layout work only)
# ---------------------------------------------------------------------------


def _prep_edges(edge_src, edge_dst, edge_attr):
    src = np.asarray(edge_src).astype(np.int64).ravel()
    dst = np.asarray(edge_dst).astype(np.int64).ravel()
    attr = np.asarray(edge_attr).astype(np.float32).ravel()

    order = np.argsort(dst, kind="stable")
    src, dst, attr = src[order], dst[order], attr[order]

    bounds = np.searchsorted(dst, np.arange(NCORES + 1) * NL)
    per_core = []
    nchunk = 1
    for c in range(NCORES):
        s_, e_ = bounds[c], bounds[c + 1]
        dl = dst[s_:e_] - c * NL
        t_id = dl >> 7                       # dst tile within core
        counts = np.bincount(t_id, minlength=TILES)
        nchunk = max(nchunk, int(np.ceil(counts.max() / 128)))
        per_core.append((src[s_:e_], dl, attr[s_:e_], t_id, counts))

    eidx_all, ind_all = [], []
    te = nchunk * 128
    for c in range(NCORES):
        s_c, dl, at, t_id, counts = per_core[c]
        starts = np.zeros(TILES, dtype=np.int64)
        starts[1:] = np.cumsum(counts)[:-1]
        slot = np.arange(len(s_c)) - starts[t_id]     # rank within its tile

        idx_flat = np.zeros((TILES, te), dtype=np.int16)
        idx_flat[t_id, slot] = s_c.astype(np.int16)
        # dma_gather idx layout: index i -> partition i%16, col i//16,
        # replicated into all 8 groups of 16 partitions.
        e16 = idx_flat.reshape(TILES, te // 16, 16).transpose(0, 2, 1)  # [T,16,te/16]
        eidx = np.tile(e16, (1, 8, 1))                                   # [T,128,te/16]

        # indicator blocks, e-major: ind[t, e, chunk, dstlocal]
        ind = np.zeros((TILES, 128, nchunk, 128), dtype=np.float32)
        ind[t_id, slot % 128, slot >> 7, dl & 127] = at
        eidx_all.append(np.ascontiguousarray(eidx))
        ind_all.append(np.ascontiguousarray(ind))
    return nchunk, eidx_all, ind_all


# ---------------------------------------------------------------------------
# Device kernel builder
# ---------------------------------------------------------------------------


def _build(nchunk):
    nc = bacc.Bacc(None, target_bir_lowering=False, debug=False, num_swdge_queues=4)
    te = nchunk * 128

    inp = {}

    def dram_in(name, shape, dt=F32):
        inp[name] = nc.dram_tensor(name, shape, dt, kind="ExternalInput")
        return inp[name]

    x = dram_in("x", [N, D])
    eidx = dram_in("eidx", [TILES, 128, te // 16], I16)
    ind = dram_in("ind", [TILES, 128, nchunk, 128], BF16)
    for l in range(2):
        dram_in(f"cw{l}", [D, D])
        dram_in(f"iwt{l}", [D, 3 * D])
        dram_in(f"owt{l}", [D, D])
        dram_in(f"ib{l}", [1, 3 * D])
        dram_in(f"ob{l}", [1, D])
        dram_in(f"bng{l}", [1, D])
        dram_in(f"bnb{l}", [1, D])
        dram_in(f"bnm{l}", [1, D])
        dram_in(f"bnv{l}", [1, D])
        dram_in(f"lng{l}", [1, D])
        dram_in(f"lnb{l}", [1, D])
    dram_in("scwt", [D, 1])
    dram_in("scb", [1, 1])

    h_out = nc.dram_tensor("h_out", [NL, D], F32, kind="ExternalOutput")
    s_out = nc.dram_tensor("s_out", [NL], F32, kind="ExternalOutput")

    with tile.TileContext(nc) as tc:
        with (
            tc.tile_pool(name="const", bufs=1) as constp,
            tc.tile_pool(name="wts", bufs=1) as wts,
            tc.tile_pool(name="rows", bufs=1) as rows,
            tc.tile_pool(name="idxp", bufs=3) as idxp,
            tc.tile_pool(name="gath", bufs=3) as gathp,
            tc.tile_pool(name="indp", bufs=3) as indp,
            tc.tile_pool(name="sb", bufs=2) as sb,
            tc.tile_pool(name="sbig", bufs=2) as sbig,
            tc.tile_pool(name="ps_acc", bufs=1, space="PSUM") as ps_acc,
            tc.tile_pool(name="ps_tp", bufs=1, space="PSUM") as ps_tp,
            tc.tile_pool(name="ps_mm", bufs=2, space="PSUM") as ps_mm,
            tc.tile_pool(name="ps_ot", bufs=2, space="PSUM") as ps_ot,
            tc.tile_pool(name="ps_row", bufs=2, space="PSUM") as ps_row,
            tc.tile_pool(name="dram", bufs=1, space="DRAM") as dramp,
        ):
            lib_inst = nc.gpsimd.load_library(_mlp_lib)

            def lib_op(inst):
                # custom Q7 ops must run after the library load; give the
                # scheduler an explicit ordering edge.
                tile.add_dep_helper(inst.ins, lib_inst.ins, sync=False,
                                    reason="gpsimd library op after load_library")
                return inst

            ident = constp.tile([128, 128], F32, tag="ident")
            make_identity(nc, ident[:])
            ones_col = constp.tile([128, 1], F32, tag="onec")
            nc.vector.memset(ones_col[:], 1.0)
            ones_row = constp.tile([1, 512], F32, tag="oner")
            nc.vector.memset(ones_row[:], 1.0)
            eps_col = constp.tile([128, 1], F32, tag="epsc")
            nc.vector.memset(eps_col[:], EPS)
            ones_blk = constp.tile([97, 64], F32, tag="oneb")
            nc.vector.memset(ones_blk[:], 1.0)

            # ---- weights/constants into SBUF ----
            def row_tile(name, w):
                t = rows.tile([1, w], F32, tag=name)
                nc.sync.dma_start(out=t[:], in_=inp[name][:])
                return t

            cw_sb, iwt_sb, owt_sb, ib_sb, ob_sb = {}, {}, {}, {}, {}
            lng_sb, lnb_sb, ac_sb, lgc_sb = {}, {}, {}, {}
            for l in range(2):
                # cw [256,256] -> [128, 2, 256] (fin-chunk a on middle axis)
                cwt = wts.tile([128, 2, D], F32, tag=f"cw{l}")
                nc.sync.dma_start(
                    out=cwt[:], in_=inp[f"cw{l}"][:].rearrange("(a p) h -> p a h", p=128)
                )
                cw_sb[l] = cwt
                iwtt = wts.tile([128, 2, 3 * D], F32, tag=f"iwt{l}")
                nc.sync.dma_start(
                    out=iwtt[:], in_=inp[f"iwt{l}"][:].rearrange("(a p) r -> p a r", p=128)
                )
                iwt_sb[l] = iwtt
                owtt = wts.tile([128, 2, D], F32, tag=f"owt{l}")
                nc.sync.dma_start(
                    out=owtt[:], in_=inp[f"owt{l}"][:].rearrange("(a p) h -> p a h", p=128)
                )
                owt_sb[l] = owtt
                ib_sb[l] = row_tile(f"ib{l}", 3 * D)
                ob_sb[l] = row_tile(f"ob{l}", D)
                lng_sb[l] = row_tile(f"lng{l}", D)
                lnb_sb[l] = row_tile(f"lnb{l}", D)

                # BatchNorm scale/bias rows: a = g*rsqrt(v+eps), c = b - m*a
                bng = row_tile(f"bng{l}", D)
                bnb = row_tile(f"bnb{l}", D)
                bnm = row_tile(f"bnm{l}", D)
                bnv = row_tile(f"bnv{l}", D)
                sdt = rows.tile([1, D], F32, tag=f"bnsd{l}")
                nc.scalar.activation(out=sdt[:], in_=bnv[:], func=AF.Sqrt, bias=eps_col[0:1, :])
                rsq = rows.tile([1, D], F32, tag=f"bnrs{l}")
                nc.vector.reciprocal(out=rsq[:], in_=sdt[:])
                a_row = rows.tile([1, D], F32, tag=f"bna{l}")
                nc.vector.tensor_tensor(out=a_row[:], in0=rsq[:], in1=bng[:], op=OP.mult)
                ma = rows.tile([1, D], F32, tag=f"bnma{l}")
                nc.vector.tensor_tensor(out=ma[:], in0=bnm[:], in1=a_row[:], op=OP.mult)
                c_row = rows.tile([1, D], F32, tag=f"bnc{l}")
                nc.vector.tensor_tensor(out=c_row[:], in0=bnb[:], in1=ma[:], op=OP.subtract)
                ac_sb[l] = (a_row, c_row)

                # fold a into conv weight columns: W''[f,h] = cw[f,h]*a[h]
                abc = wts.tile([128, D], F32, tag=f"abc{l}")
                lib_op(nc.gpsimd.partition_broadcast(out_ap=abc[:], in_ap=a_row[:]))
                for fb in range(2):
                    nc.vector.tensor_tensor(
                        out=cwt[:, fb, :], in0=cwt[:, fb, :], in1=abc[:], op=OP.mult
                    )

                # lg as per-partition column chunks [128, 2]
                lgc = wts.tile([128, 2], F32, tag=f"lgc{l}")
                for hb in range(2):
                    tp = ps_tp.tile([128, 1], F32, tag="tp")
                    # row -> column transpose as a K=1 matmul with a ones rhs
                    nc.tensor.matmul(
                        out=tp[:], lhsT=lng_sb[l][0:1, hb * 128 : (hb + 1) * 128],
                        rhs=ones_row[0:1, 0:1], start=True, stop=True,
                    )
                    nc.vector.tensor_copy(out=lgc[:, hb : hb + 1], in_=tp[:])
                lgc_sb[l] = lgc

            cwbf_sb, iwtbf_sb, owtbf_sb = {}, {}, {}
            for l in range(2):
                cb = wts.tile([128, 2, D], BF16, tag=f"cwbf{l}", name=f"cwbf{l}")
                nc.vector.tensor_copy(out=cb[:], in_=cw_sb[l][:])
                cwbf_sb[l] = cb
                ib_ = wts.tile([128, 2, 3 * D], BF16, tag=f"iwtbf{l}", name=f"iwtbf{l}")
                nc.vector.tensor_copy(out=ib_[:], in_=iwt_sb[l][:])
                iwtbf_sb[l] = ib_
                ob_ = wts.tile([128, 2, D], BF16, tag=f"owtbf{l}", name=f"owtbf{l}")
                nc.vector.tensor_copy(out=ob_[:], in_=owt_sb[l][:])
                owtbf_sb[l] = ob_
            ones_col_bf = constp.tile([128, 1], BF16, tag="onecb")
            nc.vector.memset(ones_col_bf[:], 1.0)

            scwt = wts.tile([128, 2], F32, tag="scwt")
            nc.sync.dma_start(
                out=scwt[:], in_=inp["scwt"][:].rearrange("(a p) o -> p (a o)", p=128)
            )
            scb = row_tile("scb", 1)

            ag_in = dramp.tile([NL, D], BF16)
            xbf = dramp.tile([N, D], BF16)
            xr = x[:].rearrange("(a p) d -> p a d", p=128)
            xbr = xbf[:].rearrange("(a p) d -> p a d", p=128)
            for gch in range(32):
                xc = sbig.tile([128, 8, D], BF16, tag="xprep", name="xc", bufs=3)
                nc.gpsimd.dma_start(
                    out=xc[:], in_=xr[:, gch * 8 : (gch + 1) * 8, :]
                )
                nc.sync.dma_start(
                    out=xbr[:, gch * 8 : (gch + 1) * 8, :], in_=xc[:]
                )
            ag_out = dramp.tile([N, D], BF16, addr_space="Shared")

            # ---------------- layers ----------------
            n_layers = 1 if os.environ.get("DIGCN_L0_ONLY") == "1" else 2
            for l in range(n_layers):
                table = xbf if l == 0 else ag_out
                ind_t = ind
                dt_g = BF16

                for g in range(G):
                    convT_sb = sbig.tile([128, 2, 512], BF16, tag="convT")
                    convAT_sb = sbig.tile([128, 2, 512], BF16, tag="convAT", bufs=1)
                    for t4 in range(4):
                        t = g * 4 + t4
                        idx_sb = idxp.tile([128, te // 16], I16, tag="idx")
                        nc.sync.dma_start(out=idx_sb[:], in_=eidx[t])
                        gt = gathp.tile([128, nchunk, D], dt_g, tag="gath")
                        pos = 0
                        gq = 0
                        while pos < te:
                            n_i = min(768, te - pos)
                            lib_op(nc.gpsimd.dma_gather(
                                out_ap=gt[:, pos // 128 : (pos + n_i) // 128, :],
                                in_ap=table[:],
                                idxs_ap=idx_sb[:, pos // 16 : (pos + n_i) // 16],
                                num_idxs=n_i,
                                num_idxs_reg=n_i,
                                elem_size=D,
                                queue_num=(4 * t + gq) % 4,
                            ))
                            gq += 1
                            pos += n_i
                        it = indp.tile([128, nchunk, 128], ind_t.dtype, tag="ind")
                        nc.sync.dma_start(out=it[:], in_=ind_t[t])
                        convA = ps_acc.tile([128, D], F32, tag="acc")
                        for cix in range(nchunk):
                            nc.tensor.matmul(
                                out=convA[:],
                                lhsT=it[:, cix, :],
                                rhs=gt[:, cix, :],
                                start=(cix == 0),
                                stop=(cix == nchunk - 1),
                            )
                        convA_sb = sb.tile([128, D], F32, tag="convA")
                        nc.vector.tensor_copy(out=convA_sb[:], in_=convA[:])
                        for fb in range(2):
                            tp = ps_tp.tile([128, 128], F32, tag="tp")
                            nc.tensor.transpose(
                                out=tp[:], in_=convA_sb[:, fb * 128 : (fb + 1) * 128],
                                identity=ident[:],
                            )
                            nc.vector.tensor_copy(
                                out=convAT_sb[:, fb, t4 * 128 : (t4 + 1) * 128], in_=tp[:]
                            )
                    # convT = W''^T convAT + c  (BN folded)
                    a_row, c_row = ac_sb[l]
                    for hb in range(2):
                        cps = ps_mm.tile([128, 512], F32, tag="mm")
                        for fb in range(2):
                            nc.tensor.matmul(
                                out=cps[:],
                                lhsT=cwbf_sb[l][:, fb, hb * 128 : (hb + 1) * 128],
                                rhs=convAT_sb[:, fb, :],
                                start=(fb == 0),
                                stop=False,
                            )
                        nc.tensor.matmul(
                            out=cps[:],
                            lhsT=c_row[0:1, hb * 128 : (hb + 1) * 128],
                            rhs=ones_row[:],
                            start=False,
                            stop=True,
                        )
                        nc.vector.tensor_copy(out=convT_sb[:, hb, :], in_=cps[:])

                    # ---- QKV ----
                    qkT = [sbig.tile([128, 512], BF16, tag=f"qkT{rb}", name=f"qkT{rb}", bufs=2) for rb in range(4)]
                    for rb in range(4):
                        qps = ps_mm.tile([128, 512], F32, tag="mm")
                        for hb in range(2):
                            nc.tensor.matmul(
                                out=qps[:],
                                lhsT=iwtbf_sb[l][:, hb, rb * 128 : (rb + 1) * 128],
                                rhs=convT_sb[:, hb, :],
                                start=(hb == 0),
                                stop=False,
                            )
                        nc.tensor.matmul(
                            out=qps[:],
                            lhsT=ib_sb[l][0:1, rb * 128 : (rb + 1) * 128],
                            rhs=ones_row[:],
                            start=False,
                            stop=True,
                        )
                        if rb < 2:  # Q rows get the 1/sqrt(dh) scale
                            nc.vector.tensor_scalar_mul(
                                out=qkT[rb][:], in0=qps[:], scalar1=0.125
                            )
                        else:
                            nc.vector.tensor_copy(out=qkT[rb][:], in_=qps[:])
                    v_sb = [sb.tile([128, D], BF16, tag=f"v{tb}", name=f"v{tb}") for tb in range(4)]
                    for tb in range(4):
                        vps = ps_mm.tile([128, D], F32, tag="mm")
                        for hb in range(2):
                            nc.tensor.matmul(
                                out=vps[:],
                                lhsT=convT_sb[:, hb, tb * 128 : (tb + 1) * 128],
                                rhs=iwtbf_sb[l][:, hb, 512:768],
                                start=(hb == 0),
                                stop=False,
                            )
                        nc.tensor.matmul(
                            out=vps[:],
                            lhsT=ones_row[0:1, 0:128],
                            rhs=ib_sb[l][0:1, 512:768],
                            start=False,
                            stop=True,
                        )
                        nc.vector.tensor_copy(out=v_sb[tb][:], in_=vps[:])

                    # ---- attention (transposed layouts) ----
                    oT = [ps_ot.tile([128, 512], F32, tag="oT", name=f"oT{_i}") for _i in range(2)]
                    # softmax denominators for all 4 heads accumulate into one
                    # psum bank (rows 0/32/64/96) -> one batched reciprocal.
                    srow = ps_row.tile([97, 512], F32, tag="row")
                    srow3 = ps_row.tile([1, 512], F32, tag="row", name="srow3")
                    for h in range(4):
                        qrb, off = h // 2, 64 * (h % 2)
                        kt = qkT[2 + qrb]
                        qt = qkT[qrb]
                        for kb in range(4):
                            eps_ = ps_mm.tile([128, 512], F32, tag="mm")
                            nc.tensor.matmul(
                                out=eps_[:],
                                lhsT=kt[off : off + 64, kb * 128 : (kb + 1) * 128],
                                rhs=qt[off : off + 64, :],
                                start=True,
                                stop=True,
                            )
                            et = sbig.tile([128, 512], BF16, tag="eT", bufs=3)
                            nc.scalar.activation(out=et[:], in_=eps_[:], func=AF.Exp)
                            s_dst = srow3[:] if h == 3 else srow[32 * h : 32 * h + 1, :]
                            nc.tensor.matmul(
                                out=s_dst,
                                lhsT=ones_col_bf[:],
                                rhs=et[:],
                                start=(kb == 0),
                                stop=(kb == 3),
                            )
                            nc.tensor.matmul(
                                out=oT[qrb][off : off + 64, :],
                                lhsT=v_sb[kb][:, h * 64 : (h + 1) * 64],
                                rhs=et[:],
                                start=(kb == 0),
                                stop=(kb == 3),
                            )
                    rrec = sb.tile([97, 512], F32, tag="rrec")
                    for h_ in range(3):
                        nc.vector.reciprocal(
                            out=rrec[32 * h_ : 32 * h_ + 1, :],
                            in_=srow[32 * h_ : 32 * h_ + 1, :],
                        )
                    rrec3 = sb.tile([1, 512], F32, tag="rrec3")
                    nc.vector.reciprocal(out=rrec3[:], in_=srow3[:])
                    # broadcast 1/s over the head partitions with K=1 matmuls
                    # (PE broadcast; avoids gpsimd partition_broadcast here)
                    rb2 = []
                    for p in range(2):
                        rbp = ps_mm.tile([128, 512], F32, tag="mm", name=f"rbp{p}")
                        for sub in range(2):
                            h_ = 2 * p + sub
                            if h_ == 3:
                                r_src, o_src = rrec3[:], ones_row[0:1, 0:64]
                            else:
                                r_src = rrec[32 * h_ : 32 * h_ + 1, :]
                                o_src = ones_blk[32 * h_ : 32 * h_ + 1, :]
                            nc.tensor.matmul(
                                out=rbp[64 * sub : 64 * sub + 64, :],
                                lhsT=o_src,
                                rhs=r_src,
                                start=True,
                                stop=True,
                            )
                        rbs = sbig.tile([128, 512], F32, tag=f"rb2{p}", name=f"rb2{p}", bufs=1)
                        nc.vector.tensor_copy(out=rbs[:], in_=rbp[:])
                        rb2.append(rbs)
                    stack = [sbig.tile([128, 512], BF16, tag=f"stk{p}", name=f"stk{p}", bufs=2) for p in range(2)]
                    for p in range(2):
                        nc.vector.tensor_tensor(
                            out=stack[p][:],
                            in0=oT[p][:],
                            in1=rb2[p][:],
                            op=OP.mult,
                        )

                    # ---- out-proj + residual + LayerNorm ----
                    yT = [sbig.tile([128, 512], F32, tag=f"yT{hb}", name=f"yT{hb}", bufs=1) for hb in range(2)]
                    y2 = sbig.tile([128, 512], F32, tag="y2", bufs=1)
                    lnrow = ps_row.tile([33, 512], F32, tag="row")
                    mu_ps = lnrow[0:1, :]
                    s2_ps = lnrow[32:33, :]
                    for hb in range(2):
                        aps = ps_mm.tile([128, 512], F32, tag="mm")
                        for p in range(2):
                            nc.tensor.matmul(
                                out=aps[:],
                                lhsT=owtbf_sb[l][:, p, hb * 128 : (hb + 1) * 128],
                                rhs=stack[p][:],
                                start=(p == 0),
                                stop=False,
                            )
                        nc.tensor.matmul(
                            out=aps[:],
                            lhsT=ob_sb[l][0:1, hb * 128 : (hb + 1) * 128],
                            rhs=ones_row[:],
                            start=False,
                            stop=True,
                        )
                        nc.vector.tensor_tensor(
                            out=yT[hb][:], in0=aps[:], in1=convT_sb[:, hb, :], op=OP.add
                        )
                        nc.tensor.matmul(
                            out=mu_ps, lhsT=ones_col[:], rhs=yT[hb][:],
                            start=(hb == 0), stop=(hb == 1),
                        )
                        nc.scalar.square(out=y2[:], in_=yT[hb][:])
                        nc.tensor.matmul(
                            out=s2_ps, lhsT=ones_col[:], rhs=y2[:],
                            start=(hb == 0), stop=(hb == 1),
                        )
                    mean = sb.tile([1, 512], F32, tag="mean")
                    nc.vector.tensor_scalar_mul(out=mean[:], in0=mu_ps, scalar1=1.0 / D)
                    msq = sb.tile([1, 512], F32, tag="msq")
                    nc.vector.tensor_tensor(out=msq[:], in0=mean[:], in1=mean[:], op=OP.mult)
                    var = sb.tile([1, 512], F32, tag="var")
                    nc.vector.tensor_scalar_mul(out=var[:], in0=s2_ps, scalar1=1.0 / D)
                    nc.vector.tensor_tensor(out=var[:], in0=var[:], in1=msq[:], op=OP.subtract)
                    sd = sb.tile([1, 512], F32, tag="sd")
                    nc.scalar.activation(out=sd[:], in_=var[:], func=AF.Sqrt, bias=eps_col[0:1, :])
                    rstd = sb.tile([1, 512], F32, tag="rstd")
                    nc.vector.reciprocal(out=rstd[:], in_=sd[:])
                    nbeta = sb.tile([1, 512], F32, tag="nbeta")
                    nc.vector.tensor_tensor(out=nbeta[:], in0=mean[:], in1=rstd[:], op=OP.mult)
                    nc.vector.tensor_scalar_mul(out=nbeta[:], in0=nbeta[:], scalar1=-1.0)
                    ab = sbig.tile([128, 512], F32, tag="ab", bufs=1)
                    lib_op(nc.gpsimd.partition_broadcast(out_ap=ab[:], in_ap=rstd[:]))

                    hT = [sbig.tile([128, 512], F32, tag=f"hT{hb}", name=f"hT{hb}", bufs=1) for hb in range(2)]
                    for hb in range(2):
                        t2p = ps_mm.tile([128, 512], F32, tag="mm")
                        nc.tensor.matmul(
                            out=t2p[:],
                            lhsT=lng_sb[l][0:1, hb * 128 : (hb + 1) * 128],
                            rhs=nbeta[:],
                            start=True,
                            stop=False,
                        )
                        nc.tensor.matmul(
                            out=t2p[:],
                            lhsT=lnb_sb[l][0:1, hb * 128 : (hb + 1) * 128],
                            rhs=ones_row[:],
                            start=False,
                            stop=True,
                        )
                        nc.vector.tensor_tensor(
                            out=hT[hb][:], in0=yT[hb][:], in1=ab[:], op=OP.mult
                        )
                        nc.vector.tensor_scalar_mul(
                            out=hT[hb][:], in0=hT[hb][:], scalar1=lgc_sb[l][:, hb : hb + 1]
                        )
                        nc.vector.tensor_tensor(
                            out=hT[hb][:], in0=hT[hb][:], in1=t2p[:], op=OP.add
                        )
                        if l == 0:
                            nc.scalar.activation(out=hT[hb][:], in_=hT[hb][:], func=AF.Relu)

                    # ---- layer epilogue per graph ----
                    for tb in range(4):
                        hn = sb.tile(
                            [128, D],
                            BF16 if (l == 0 and n_layers == 2) else F32,
                            tag="hn",
                        )
                        for hb in range(2):
                            tp = ps_tp.tile([128, 128], F32, tag="tp")
                            nc.tensor.transpose(
                                out=tp[:], in_=hT[hb][:, tb * 128 : (tb + 1) * 128],
                                identity=ident[:],
                            )
                            nc.vector.tensor_copy(
                                out=hn[:, hb * 128 : (hb + 1) * 128], in_=tp[:]
                            )
                        row0 = g * S + tb * 128
                        if l == 0 and n_layers == 1:
                            nc.sync.dma_start(
                                out=h_out[row0 : row0 + 128, :], in_=hn[:]
                            )
                        elif l == 0:
                            nc.sync.dma_start(
                                out=ag_in[row0 : row0 + 128, :], in_=hn[:]
                            )
                        else:
                            nc.sync.dma_start(
                                out=h_out[row0 : row0 + 128, :], in_=hn[:]
                            )
                    if l == 1:
                        scp = ps_row.tile([1, 512], F32, tag="row")
                        for hb in range(2):
                            nc.tensor.matmul(
                                out=scp[:],
                                lhsT=scwt[:, hb : hb + 1],
                                rhs=hT[hb][:],
                                start=(hb == 0),
                                stop=(hb == 1),
                            )
                        scs = sb.tile([1, 512], F32, tag="scs")
                        nc.vector.tensor_scalar(
                            out=scs[:], in0=scp[:], scalar1=scb[:], scalar2=None,
                            op0=OP.add,
                        )
                        nc.sync.dma_start(out=s_out[g * S : (g + 1) * S], in_=scs[0:1, :])

                if l == 0 and n_layers == 2:
                    nc.gpsimd.collective_compute(
                        "AllGather",
                        OP.bypass,
                        replica_groups=[list(range(NCORES))],
                        ins=[ag_in[:].opt()],
                        outs=[ag_out[:].opt()],
                    )
    nc.compile()
    return nc


_CACHE = {}


def kernel(**inputs):
    x = np.ascontiguousarray(np.asarray(inputs["x"], dtype=np.float32))
    nchunk, eidx_all, ind_all = _prep_edges(
        inputs["edge_src"], inputs["edge_dst"], inputs["edge_attr"]
    )

    if nchunk not in _CACHE:
        _CACHE[nchunk] = _build(nchunk)
    nc = _CACHE[nchunk]

    base = {"x": x}
    for l in range(2):
        base[f"cw{l}"] = np.ascontiguousarray(np.asarray(inputs[f"conv_w{l}"], np.float32))
        base[f"iwt{l}"] = np.ascontiguousarray(
            np.asarray(inputs[f"attn_in_w{l}"], np.float32).T
        )
        base[f"owt{l}"] = np.ascontiguousarray(
            np.asarray(inputs[f"attn_out_w{l}"], np.float32).T
        )
        base[f"ib{l}"] = np.asarray(inputs[f"attn_in_b{l}"], np.float32).reshape(1, -1)
        base[f"ob{l}"] = np.asarray(inputs[f"attn_out_b{l}"], np.float32).reshape(1, -1)
        base[f"bng{l}"] = np.asarray(inputs[f"bn_g{l}"], np.float32).reshape(1, -1)
        base[f"bnb{l}"] = np.asarray(inputs[f"bn_b{l}"], np.float32).reshape(1, -1)
        base[f"bnm{l}"] = np.asarray(inputs[f"bn_m{l}"], np.float32).reshape(1, -1)
        base[f"bnv{l}"] = np.asarray(inputs[f"bn_v{l}"], np.float32).reshape(1, -1)
        base[f"lng{l}"] = np.asarray(inputs[f"ln_g{l}"], np.float32).reshape(1, -1)
        base[f"lnb{l}"] = np.asarray(inputs[f"ln_b{l}"], np.float32).reshape(1, -1)
    base["scwt"] = np.ascontiguousarray(np.asarray(inputs["score_w"], np.float32).T)
    base["scb"] = np.asarray(inputs["score_b"], np.float32).reshape(1, 1)

    in_maps = []
    for c in range(NCORES):
        m = dict(base)
        m["eidx"] = eidx_all[c]
        m["ind"] = ind_all[c].astype(NP_BF16)
        in_maps.append(m)

    res = run_bass_kernel_spmd(nc, in_maps, list(range(NCORES)))
    kernel.last_results = res

    h = np.concatenate([res.results[c]["h_out"] for c in range(NCORES)], axis=0)
    scores = np.concatenate([res.results[c]["s_out"] for c in range(NCORES)], axis=0)
    return h, scores


# revision 27
# speedup vs baseline: 1.1047x; 1.1047x over previous
"""DiGCN (2-layer DIGCNConv + BatchNorm + per-graph MHA + LayerNorm) on 8
Trainium2 NeuronCores.

Strategy (sharding_hint: data-parallel over graphs):
  - 64 graphs of 512 nodes -> core c owns graphs 8c..8c+7 (4096 dst nodes).
  - Edges are partitioned by dst core and CSR-sorted by dst on the host;
    per 128-dst tile they are padded to a uniform chunk grid.  The
    segment-sum  conv[d] += attr_e * hW[src_e]  is computed as a chain of
    PE matmuls against host-built "indicator" blocks Ind[e, d] =
    attr_e * (dst_e == d), which also folds in edge_attr.
  - conv = A @ (h @ W) is commuted to (A @ h) @ W so layer 0 gathers rows
    of x directly (fp32, no table build).  Layer 1 gathers from an
    AllGather'ed bf16 copy of h1.
  - Attention is computed in transposed layouts (scoresT = K Q^T) so no
    512x512 transposes are needed; softmax denominators come from
    ones-vector matmuls, and normalization uses gpsimd partition_broadcast.
"""

import os
import sys

for _p in ("/opt/trn_rl_repo", "/root/.axon_site/_ro/trn_rl_repo"):
    if os.path.isdir(_p) and _p not in sys.path:
        sys.path.insert(0, _p)

import numpy as np

import concourse.bacc as bacc
import concourse.bass as bass
import concourse.tile as tile
from concourse import mybir
from concourse.bass_utils import run_bass_kernel_spmd
from concourse.library_config import mlp as _mlp_lib
from concourse.masks import make_identity

F32 = mybir.dt.float32
BF16 = mybir.dt.bfloat16
I16 = mybir.dt.int16
NP_BF16 = mybir.dt.np(BF16)
AF = mybir.ActivationFunctionType
OP = mybir.AluOpType

N = 32768
E = 524288
D = 256          # NFEAT == NHID
HEADS = 4
DH = 64
NCORES = 8
NL = N // NCORES         # 4096 nodes per core
G = 8                    # graphs per core
S = 512                  # nodes per graph
TILES = NL // 128        # 32 dst tiles per core
EPS = 1e-5

# ---------------------------------------------------------------------------
# Host-side edge preprocessing (index/layout work only)
# ---------------------------------------------------------------------------


def _prep_edges(edge_src, edge_dst, edge_attr):
    src = np.asarray(edge_src).astype(np.int64).ravel()
    dst = np.asarray(edge_dst).astype(np.int64).ravel()
    attr = np.asarray(edge_attr).astype(np.float32).ravel()

    order = np.argsort(dst, kind="stable")
    src, dst, attr = src[order], dst[order], attr[order]

    bounds = np.searchsorted(dst, np.arange(NCORES + 1) * NL)
    per_core = []
    nchunk = 1
    for c in range(NCORES):
        s_, e_ = bounds[c], bounds[c + 1]
        dl = dst[s_:e_] - c * NL
        t_id = dl >> 7                       # dst tile within core
        counts = np.bincount(t_id, minlength=TILES)
        nchunk = max(nchunk, int(np.ceil(counts.max() / 128)))
        per_core.append((src[s_:e_], dl, attr[s_:e_], t_id, counts))

    eidx_all, ind_all = [], []
    te = nchunk * 128
    for c in range(NCORES):
        s_c, dl, at, t_id, counts = per_core[c]
        starts = np.zeros(TILES, dtype=np.int64)
        starts[1:] = np.cumsum(counts)[:-1]
        slot = np.arange(len(s_c)) - starts[t_id]     # rank within its tile

        idx_flat = np.zeros((TILES, te), dtype=np.int16)
        idx_flat[t_id, slot] = s_c.astype(np.int16)
        # dma_gather idx layout: index i -> partition i%16, col i//16,
        # replicated into all 8 groups of 16 partitions.
        e16 = idx_flat.reshape(TILES, te // 16, 16).transpose(0, 2, 1)  # [T,16,te/16]
        eidx = np.tile(e16, (1, 8, 1))                                   # [T,128,te/16]

        # indicator blocks, e-major: ind[t, e, chunk, dstlocal]
        ind = np.zeros((TILES, 128, nchunk, 128), dtype=np.float32)
        ind[t_id, slot % 128, slot >> 7, dl & 127] = at
        eidx_all.append(np.ascontiguousarray(eidx))
        ind_all.append(np.ascontiguousarray(ind))
    return nchunk, eidx_all, ind_all


# ---------------------------------------------------------------------------
# Device kernel builder
# ---------------------------------------------------------------------------


def _build(nchunk):
    nc = bacc.Bacc(None, target_bir_lowering=False, debug=False, num_swdge_queues=4)
    te = nchunk * 128

    inp = {}

    def dram_in(name, shape, dt=F32):
        inp[name] = nc.dram_tensor(name, shape, dt, kind="ExternalInput")
        return inp[name]

    x = dram_in("x", [N, D])
    eidx = dram_in("eidx", [TILES, 128, te // 16], I16)
    ind = dram_in("ind", [TILES, 128, nchunk, 128], BF16)
    for l in range(2):
        dram_in(f"cw{l}", [D, D])
        dram_in(f"iwt{l}", [D, 3 * D])
        dram_in(f"owt{l}", [D, D])
        dram_in(f"ib{l}", [1, 3 * D])
        dram_in(f"ob{l}", [1, D])
        dram_in(f"bng{l}", [1, D])
        dram_in(f"bnb{l}", [1, D])
        dram_in(f"bnm{l}", [1, D])
        dram_in(f"bnv{l}", [1, D])
        dram_in(f"lng{l}", [1, D])
        dram_in(f"lnb{l}", [1, D])
    dram_in("scwt", [D, 1])
    dram_in("scb", [1, 1])

    h_out = nc.dram_tensor("h_out", [NL, D], F32, kind="ExternalOutput")
    s_out = nc.dram_tensor("s_out", [NL], F32, kind="ExternalOutput")

    with tile.TileContext(nc) as tc:
        with (
            tc.tile_pool(name="const", bufs=1) as constp,
            tc.tile_pool(name="wts", bufs=1) as wts,
            tc.tile_pool(name="rows", bufs=1) as rows,
            tc.tile_pool(name="idxp", bufs=3) as idxp,
            tc.tile_pool(name="gath", bufs=3) as gathp,
            tc.tile_pool(name="indp", bufs=3) as indp,
            tc.tile_pool(name="sb", bufs=2) as sb,
            tc.tile_pool(name="sbig", bufs=2) as sbig,
            tc.tile_pool(name="ps_acc", bufs=1, space="PSUM") as ps_acc,
            tc.tile_pool(name="ps_tp", bufs=1, space="PSUM") as ps_tp,
            tc.tile_pool(name="ps_mm", bufs=2, space="PSUM") as ps_mm,
            tc.tile_pool(name="ps_ot", bufs=2, space="PSUM") as ps_ot,
            tc.tile_pool(name="ps_row", bufs=2, space="PSUM") as ps_row,
            tc.tile_pool(name="dram", bufs=1, space="DRAM") as dramp,
        ):
            lib_inst = nc.gpsimd.load_library(_mlp_lib)

            def lib_op(inst):
                # custom Q7 ops must run after the library load; give the
                # scheduler an explicit ordering edge.
                tile.add_dep_helper(inst.ins, lib_inst.ins, sync=False,
                                    reason="gpsimd library op after load_library")
                return inst

            ident = constp.tile([128, 128], F32, tag="ident")
            make_identity(nc, ident[:])
            ones_col = constp.tile([128, 1], F32, tag="onec")
            nc.vector.memset(ones_col[:], 1.0)
            ones_row = constp.tile([1, 512], F32, tag="oner")
            nc.vector.memset(ones_row[:], 1.0)
            eps_col = constp.tile([128, 1], F32, tag="epsc")
            nc.vector.memset(eps_col[:], EPS)
            ones_blk = constp.tile([97, 64], F32, tag="oneb")
            nc.vector.memset(ones_blk[:], 1.0)

            # ---- weights/constants into SBUF ----
            def row_tile(name, w):
                t = rows.tile([1, w], F32, tag=name)
                nc.sync.dma_start(out=t[:], in_=inp[name][:])
                return t

            cw_sb, iwt_sb, owt_sb, ib_sb, ob_sb = {}, {}, {}, {}, {}
            lng_sb, lnb_sb, ac_sb, lgc_sb = {}, {}, {}, {}
            for l in range(2):
                # cw [256,256] -> [128, 2, 256] (fin-chunk a on middle axis)
                cwt = wts.tile([128, 2, D], F32, tag=f"cw{l}")
                nc.sync.dma_start(
                    out=cwt[:], in_=inp[f"cw{l}"][:].rearrange("(a p) h -> p a h", p=128)
                )
                cw_sb[l] = cwt
                iwtt = wts.tile([128, 2, 3 * D], F32, tag=f"iwt{l}")
                nc.sync.dma_start(
                    out=iwtt[:], in_=inp[f"iwt{l}"][:].rearrange("(a p) r -> p a r", p=128)
                )
                iwt_sb[l] = iwtt
                owtt = wts.tile([128, 2, D], F32, tag=f"owt{l}")
                nc.sync.dma_start(
                    out=owtt[:], in_=inp[f"owt{l}"][:].rearrange("(a p) h -> p a h", p=128)
                )
                owt_sb[l] = owtt
                ib_sb[l] = row_tile(f"ib{l}", 3 * D)
                ob_sb[l] = row_tile(f"ob{l}", D)
                lng_sb[l] = row_tile(f"lng{l}", D)
                lnb_sb[l] = row_tile(f"lnb{l}", D)

                # BatchNorm scale/bias rows: a = g*rsqrt(v+eps), c = b - m*a
                bng = row_tile(f"bng{l}", D)
                bnb = row_tile(f"bnb{l}", D)
                bnm = row_tile(f"bnm{l}", D)
                bnv = row_tile(f"bnv{l}", D)
                sdt = rows.tile([1, D], F32, tag=f"bnsd{l}")
                nc.scalar.activation(out=sdt[:], in_=bnv[:], func=AF.Sqrt, bias=eps_col[0:1, :])
                rsq = rows.tile([1, D], F32, tag=f"bnrs{l}")
                nc.vector.reciprocal(out=rsq[:], in_=sdt[:])
                a_row = rows.tile([1, D], F32, tag=f"bna{l}")
                nc.vector.tensor_tensor(out=a_row[:], in0=rsq[:], in1=bng[:], op=OP.mult)
                ma = rows.tile([1, D], F32, tag=f"bnma{l}")
                nc.vector.tensor_tensor(out=ma[:], in0=bnm[:], in1=a_row[:], op=OP.mult)
                c_row = rows.tile([1, D], F32, tag=f"bnc{l}")
                nc.vector.tensor_tensor(out=c_row[:], in0=bnb[:], in1=ma[:], op=OP.subtract)
                ac_sb[l] = (a_row, c_row)

                # fold a into conv weight columns: W''[f,h] = cw[f,h]*a[h]
                abc = wts.tile([128, D], F32, tag=f"abc{l}")
                lib_op(nc.gpsimd.partition_broadcast(out_ap=abc[:], in_ap=a_row[:]))
                for fb in range(2):
                    nc.vector.tensor_tensor(
                        out=cwt[:, fb, :], in0=cwt[:, fb, :], in1=abc[:], op=OP.mult
                    )

                # lg as per-partition column chunks [128, 2]
                lgc = wts.tile([128, 2], F32, tag=f"lgc{l}")
                for hb in range(2):
                    tp = ps_tp.tile([128, 1], F32, tag="tp")
                    # row -> column transpose as a K=1 matmul with a ones rhs
                    nc.tensor.matmul(
                        out=tp[:], lhsT=lng_sb[l][0:1, hb * 128 : (hb + 1) * 128],
                        rhs=ones_row[0:1, 0:1], start=True, stop=True,
                    )
                    nc.vector.tensor_copy(out=lgc[:, hb : hb + 1], in_=tp[:])
                lgc_sb[l] = lgc

            cwbf_sb, iwtbf_sb, owtbf_sb = {}, {}, {}
            for l in range(2):
                cb = wts.tile([128, 2, D], BF16, tag=f"cwbf{l}", name=f"cwbf{l}")
                nc.vector.tensor_copy(out=cb[:], in_=cw_sb[l][:])
                cwbf_sb[l] = cb
                ib_ = wts.tile([128, 2, 3 * D], BF16, tag=f"iwtbf{l}", name=f"iwtbf{l}")
                nc.vector.tensor_copy(out=ib_[:], in_=iwt_sb[l][:])
                iwtbf_sb[l] = ib_
                ob_ = wts.tile([128, 2, D], BF16, tag=f"owtbf{l}", name=f"owtbf{l}")
                nc.vector.tensor_copy(out=ob_[:], in_=owt_sb[l][:])
                owtbf_sb[l] = ob_
            ones_col_bf = constp.tile([128, 1], BF16, tag="onecb")
            nc.vector.memset(ones_col_bf[:], 1.0)

            scwt = wts.tile([128, 2], F32, tag="scwt")
            nc.sync.dma_start(
                out=scwt[:], in_=inp["scwt"][:].rearrange("(a p) o -> p (a o)", p=128)
            )
            scb = row_tile("scb", 1)

            ag_in = dramp.tile([NL, D], BF16)
            xbf = dramp.tile([N, D], BF16)
            # one DRAM->DRAM SWDGE cast DMA builds the bf16 gather table
            # (no SBUF bounce; halves the prologue traffic)
            nc.gpsimd.dma_start(out=xbf[:], in_=x[:])
            ag_out = dramp.tile([N, D], BF16, addr_space="Shared")

            # ---------------- layers ----------------
            n_layers = 1 if os.environ.get("DIGCN_L0_ONLY") == "1" else 2
            for l in range(n_layers):
                table = xbf if l == 0 else ag_out
                ind_t = ind
                dt_g = BF16

                for g in range(G):
                    convT_sb = sbig.tile([128, 2, 512], BF16, tag="convT")
                    convAT_sb = sbig.tile([128, 2, 512], BF16, tag="convAT", bufs=1)
                    for t4 in range(4):
                        t = g * 4 + t4
                        idx_sb = idxp.tile([128, te // 16], I16, tag="idx")
                        nc.sync.dma_start(out=idx_sb[:], in_=eidx[t])
                        gt = gathp.tile([128, nchunk, D], dt_g, tag="gath")
                        pos = 0
                        gq = 0
                        while pos < te:
                            n_i = min(768, te - pos)
                            lib_op(nc.gpsimd.dma_gather(
                                out_ap=gt[:, pos // 128 : (pos + n_i) // 128, :],
                                in_ap=table[:],
                                idxs_ap=idx_sb[:, pos // 16 : (pos + n_i) // 16],
                                num_idxs=n_i,
                                num_idxs_reg=n_i,
                                elem_size=D,
                                queue_num=(4 * t + gq) % 4,
                            ))
                            gq += 1
                            pos += n_i
                        it = indp.tile([128, nchunk, 128], ind_t.dtype, tag="ind")
                        nc.sync.dma_start(out=it[:], in_=ind_t[t])
                        convA = ps_acc.tile([128, D], F32, tag="acc")
                        for cix in range(nchunk):
                            nc.tensor.matmul(
                                out=convA[:],
                                lhsT=it[:, cix, :],
                                rhs=gt[:, cix, :],
                                start=(cix == 0),
                                stop=(cix == nchunk - 1),
                            )
                        convA_sb = sb.tile([128, D], F32, tag="convA")
                        nc.vector.tensor_copy(out=convA_sb[:], in_=convA[:])
                        for fb in range(2):
                            tp = ps_tp.tile([128, 128], F32, tag="tp")
                            nc.tensor.transpose(
                                out=tp[:], in_=convA_sb[:, fb * 128 : (fb + 1) * 128],
                                identity=ident[:],
                            )
                            nc.vector.tensor_copy(
                                out=convAT_sb[:, fb, t4 * 128 : (t4 + 1) * 128], in_=tp[:]
                            )
                    # convT = W''^T convAT + c  (BN folded)
                    a_row, c_row = ac_sb[l]
                    for hb in range(2):
                        cps = ps_mm.tile([128, 512], F32, tag="mm")
                        for fb in range(2):
                            nc.tensor.matmul(
                                out=cps[:],
                                lhsT=cwbf_sb[l][:, fb, hb * 128 : (hb + 1) * 128],
                                rhs=convAT_sb[:, fb, :],
                                start=(fb == 0),
                                stop=False,
                            )
                        nc.tensor.matmul(
                            out=cps[:],
                            lhsT=c_row[0:1, hb * 128 : (hb + 1) * 128],
                            rhs=ones_row[:],
                            start=False,
                            stop=True,
                        )
                        nc.vector.tensor_copy(out=convT_sb[:, hb, :], in_=cps[:])

                    # ---- QKV ----
                    qkT = [sbig.tile([128, 512], BF16, tag=f"qkT{rb}", name=f"qkT{rb}", bufs=1) for rb in range(4)]
                    for rb in range(4):
                        qps = ps_mm.tile([128, 512], F32, tag="mm")
                        for hb in range(2):
                            nc.tensor.matmul(
                                out=qps[:],
                                lhsT=iwtbf_sb[l][:, hb, rb * 128 : (rb + 1) * 128],
                                rhs=convT_sb[:, hb, :],
                                start=(hb == 0),
                                stop=False,
                            )
                        nc.tensor.matmul(
                            out=qps[:],
                            lhsT=ib_sb[l][0:1, rb * 128 : (rb + 1) * 128],
                            rhs=ones_row[:],
                            start=False,
                            stop=True,
                        )
                        if rb < 2:  # Q rows get the 1/sqrt(dh) scale
                            nc.vector.tensor_scalar_mul(
                                out=qkT[rb][:], in0=qps[:], scalar1=0.125
                            )
                        else:
                            nc.vector.tensor_copy(out=qkT[rb][:], in_=qps[:])
                    v_sb = [sb.tile([128, D], BF16, tag=f"v{tb}", name=f"v{tb}") for tb in range(4)]
                    for tb in range(4):
                        vps = ps_mm.tile([128, D], F32, tag="mm")
                        for hb in range(2):
                            nc.tensor.matmul(
                                out=vps[:],
                                lhsT=convT_sb[:, hb, tb * 128 : (tb + 1) * 128],
                                rhs=iwtbf_sb[l][:, hb, 512:768],
                                start=(hb == 0),
                                stop=False,
                            )
                        nc.tensor.matmul(
                            out=vps[:],
                            lhsT=ones_row[0:1, 0:128],
                            rhs=ib_sb[l][0:1, 512:768],
                            start=False,
                            stop=True,
                        )
                        nc.vector.tensor_copy(out=v_sb[tb][:], in_=vps[:])

                    # ---- attention (transposed layouts) ----
                    oT = [ps_ot.tile([128, 512], F32, tag="oT", name=f"oT{_i}") for _i in range(2)]
                    # softmax denominators for all 4 heads accumulate into one
                    # psum bank (rows 0/32/64/96) -> one batched reciprocal.
                    srow = ps_row.tile([97, 512], F32, tag="row")
                    srow3 = ps_row.tile([1, 512], F32, tag="row", name="srow3")
                    for h in range(4):
                        qrb, off = h // 2, 64 * (h % 2)
                        kt = qkT[2 + qrb]
                        qt = qkT[qrb]
                        for kb in range(4):
                            eps_ = ps_mm.tile([128, 512], F32, tag="mm")
                            nc.tensor.matmul(
                                out=eps_[:],
                                lhsT=kt[off : off + 64, kb * 128 : (kb + 1) * 128],
                                rhs=qt[off : off + 64, :],
                                start=True,
                                stop=True,
                            )
                            et = sbig.tile([128, 512], BF16, tag="eT")
                            nc.scalar.activation(out=et[:], in_=eps_[:], func=AF.Exp)
                            s_dst = srow3[:] if h == 3 else srow[32 * h : 32 * h + 1, :]
                            nc.tensor.matmul(
                                out=s_dst,
                                lhsT=ones_col_bf[:],
                                rhs=et[:],
                                start=(kb == 0),
                                stop=(kb == 3),
                            )
                            nc.tensor.matmul(
                                out=oT[qrb][off : off + 64, :],
                                lhsT=v_sb[kb][:, h * 64 : (h + 1) * 64],
                                rhs=et[:],
                                start=(kb == 0),
                                stop=(kb == 3),
                            )
                    rrec = sb.tile([97, 512], F32, tag="rrec")
                    for h_ in range(3):
                        nc.vector.reciprocal(
                            out=rrec[32 * h_ : 32 * h_ + 1, :],
                            in_=srow[32 * h_ : 32 * h_ + 1, :],
                        )
                    rrec3 = sb.tile([1, 512], F32, tag="rrec3")
                    nc.vector.reciprocal(out=rrec3[:], in_=srow3[:])
                    # broadcast 1/s over the head partitions with K=1 matmuls
                    # (PE broadcast; avoids gpsimd partition_broadcast here)
                    rb2 = []
                    for p in range(2):
                        rbp = ps_mm.tile([128, 512], F32, tag="mm", name=f"rbp{p}")
                        for sub in range(2):
                            h_ = 2 * p + sub
                            if h_ == 3:
                                r_src, o_src = rrec3[:], ones_row[0:1, 0:64]
                            else:
                                r_src = rrec[32 * h_ : 32 * h_ + 1, :]
                                o_src = ones_blk[32 * h_ : 32 * h_ + 1, :]
                            nc.tensor.matmul(
                                out=rbp[64 * sub : 64 * sub + 64, :],
                                lhsT=o_src,
                                rhs=r_src,
                                start=True,
                                stop=True,
                            )
                        rbs = sbig.tile([128, 512], F32, tag=f"rb2{p}", name=f"rb2{p}", bufs=1)
                        nc.vector.tensor_copy(out=rbs[:], in_=rbp[:])
                        rb2.append(rbs)
                    stack = [sbig.tile([128, 512], BF16, tag=f"stk{p}", name=f"stk{p}", bufs=1) for p in range(2)]
                    for p in range(2):
                        nc.vector.tensor_tensor(
                            out=stack[p][:],
                            in0=oT[p][:],
                            in1=rb2[p][:],
                            op=OP.mult,
                        )

                    # ---- out-proj + residual + LayerNorm ----
                    yT = [sbig.tile([128, 512], F32, tag=f"yT{hb}", name=f"yT{hb}", bufs=1) for hb in range(2)]
                    y2 = sbig.tile([128, 512], F32, tag="y2", bufs=1)
                    lnrow = ps_row.tile([33, 512], F32, tag="row")
                    mu_ps = lnrow[0:1, :]
                    s2_ps = lnrow[32:33, :]
                    for hb in range(2):
                        aps = ps_mm.tile([128, 512], F32, tag="mm")
                        for p in range(2):
                            nc.tensor.matmul(
                                out=aps[:],
                                lhsT=owtbf_sb[l][:, p, hb * 128 : (hb + 1) * 128],
                                rhs=stack[p][:],
                                start=(p == 0),
                                stop=False,
                            )
                        nc.tensor.matmul(
                            out=aps[:],
                            lhsT=ob_sb[l][0:1, hb * 128 : (hb + 1) * 128],
                            rhs=ones_row[:],
                            start=False,
                            stop=True,
                        )
                        nc.vector.tensor_tensor(
                            out=yT[hb][:], in0=aps[:], in1=convT_sb[:, hb, :], op=OP.add
                        )
                        nc.tensor.matmul(
                            out=mu_ps, lhsT=ones_col[:], rhs=yT[hb][:],
                            start=(hb == 0), stop=(hb == 1),
                        )
                        nc.scalar.square(out=y2[:], in_=yT[hb][:])
                        nc.tensor.matmul(
                            out=s2_ps, lhsT=ones_col[:], rhs=y2[:],
                            start=(hb == 0), stop=(hb == 1),
                        )
                    mean = sb.tile([1, 512], F32, tag="mean")
                    nc.vector.tensor_scalar_mul(out=mean[:], in0=mu_ps, scalar1=1.0 / D)
                    msq = sb.tile([1, 512], F32, tag="msq")
                    nc.vector.tensor_tensor(out=msq[:], in0=mean[:], in1=mean[:], op=OP.mult)
                    var = sb.tile([1, 512], F32, tag="var")
                    nc.vector.tensor_scalar_mul(out=var[:], in0=s2_ps, scalar1=1.0 / D)
                    nc.vector.tensor_tensor(out=var[:], in0=var[:], in1=msq[:], op=OP.subtract)
                    sd = sb.tile([1, 512], F32, tag="sd")
                    nc.scalar.activation(out=sd[:], in_=var[:], func=AF.Sqrt, bias=eps_col[0:1, :])
                    rstd = sb.tile([1, 512], F32, tag="rstd")
                    nc.vector.reciprocal(out=rstd[:], in_=sd[:])
                    nbeta = sb.tile([1, 512], F32, tag="nbeta")
                    nc.vector.tensor_tensor(out=nbeta[:], in0=mean[:], in1=rstd[:], op=OP.mult)
                    nc.vector.tensor_scalar_mul(out=nbeta[:], in0=nbeta[:], scalar1=-1.0)
                    ab = sbig.tile([128, 512], F32, tag="ab", bufs=1)
                    lib_op(nc.gpsimd.partition_broadcast(out_ap=ab[:], in_ap=rstd[:]))

                    hT = [sbig.tile([128, 512], F32, tag=f"hT{hb}", name=f"hT{hb}", bufs=1) for hb in range(2)]
                    for hb in range(2):
                        t2p = ps_mm.tile([128, 512], F32, tag="mm")
                        nc.tensor.matmul(
                            out=t2p[:],
                            lhsT=lng_sb[l][0:1, hb * 128 : (hb + 1) * 128],
                            rhs=nbeta[:],
                            start=True,
                            stop=False,
                        )
                        nc.tensor.matmul(
                            out=t2p[:],
                            lhsT=lnb_sb[l][0:1, hb * 128 : (hb + 1) * 128],
                            rhs=ones_row[:],
                            start=False,
                            stop=True,
                        )
                        nc.vector.tensor_tensor(
                            out=hT[hb][:], in0=yT[hb][:], in1=ab[:], op=OP.mult
                        )
                        nc.vector.tensor_scalar_mul(
                            out=hT[hb][:], in0=hT[hb][:], scalar1=lgc_sb[l][:, hb : hb + 1]
                        )
                        nc.vector.tensor_tensor(
                            out=hT[hb][:], in0=hT[hb][:], in1=t2p[:], op=OP.add
                        )
                        if l == 0:
                            nc.scalar.activation(out=hT[hb][:], in_=hT[hb][:], func=AF.Relu)

                    # ---- layer epilogue per graph ----
                    for tb in range(4):
                        hn = sb.tile(
                            [128, D],
                            BF16 if (l == 0 and n_layers == 2) else F32,
                            tag="hn",
                        )
                        for hb in range(2):
                            tp = ps_tp.tile([128, 128], F32, tag="tp")
                            nc.tensor.transpose(
                                out=tp[:], in_=hT[hb][:, tb * 128 : (tb + 1) * 128],
                                identity=ident[:],
                            )
                            nc.vector.tensor_copy(
                                out=hn[:, hb * 128 : (hb + 1) * 128], in_=tp[:]
                            )
                        row0 = g * S + tb * 128
                        if l == 0 and n_layers == 1:
                            nc.sync.dma_start(
                                out=h_out[row0 : row0 + 128, :], in_=hn[:]
                            )
                        elif l == 0:
                            nc.sync.dma_start(
                                out=ag_in[row0 : row0 + 128, :], in_=hn[:]
                            )
                        else:
                            nc.sync.dma_start(
                                out=h_out[row0 : row0 + 128, :], in_=hn[:]
                            )
                    if l == 1:
                        scp = ps_row.tile([1, 512], F32, tag="row")
                        for hb in range(2):
                            nc.tensor.matmul(
                                out=scp[:],
                                lhsT=scwt[:, hb : hb + 1],
                                rhs=hT[hb][:],
                                start=(hb == 0),
                                stop=(hb == 1),
                            )
                        scs = sb.tile([1, 512], F32, tag="scs")
                        nc.vector.tensor_scalar(
                            out=scs[:], in0=scp[:], scalar1=scb[:], scalar2=None,
                            op0=OP.add,
                        )
                        nc.sync.dma_start(out=s_out[g * S : (g + 1) * S], in_=scs[0:1, :])

                if l == 0 and n_layers == 2:
                    nc.gpsimd.collective_compute(
                        "AllGather",
                        OP.bypass,
                        replica_groups=[list(range(NCORES))],
                        ins=[ag_in[:].opt()],
                        outs=[ag_out[:].opt()],
                    )
    nc.compile()
    return nc


_CACHE = {}


def kernel(**inputs):
    x = np.ascontiguousarray(np.asarray(inputs["x"], dtype=np.float32))
    nchunk, eidx_all, ind_all = _prep_edges(
        inputs["edge_src"], inputs["edge_dst"], inputs["edge_attr"]
    )

    if nchunk not in _CACHE:
        _CACHE[nchunk] = _build(nchunk)
    nc = _CACHE[nchunk]

    base = {"x": x}
    for l in range(2):
        base[f"cw{l}"] = np.ascontiguousarray(np.asarray(inputs[f"conv_w{l}"], np.float32))
        base[f"iwt{l}"] = np.ascontiguousarray(
            np.asarray(inputs[f"attn_in_w{l}"], np.float32).T
        )
        base[f"owt{l}"] = np.ascontiguousarray(
            np.asarray(inputs[f"attn_out_w{l}"], np.float32).T
        )
        base[f"ib{l}"] = np.asarray(inputs[f"attn_in_b{l}"], np.float32).reshape(1, -1)
        base[f"ob{l}"] = np.asarray(inputs[f"attn_out_b{l}"], np.float32).reshape(1, -1)
        base[f"bng{l}"] = np.asarray(inputs[f"bn_g{l}"], np.float32).reshape(1, -1)
        base[f"bnb{l}"] = np.asarray(inputs[f"bn_b{l}"], np.float32).reshape(1, -1)
        base[f"bnm{l}"] = np.asarray(inputs[f"bn_m{l}"], np.float32).reshape(1, -1)
        base[f"bnv{l}"] = np.asarray(inputs[f"bn_v{l}"], np.float32).reshape(1, -1)
        base[f"lng{l}"] = np.asarray(inputs[f"ln_g{l}"], np.float32).reshape(1, -1)
        base[f"lnb{l}"] = np.asarray(inputs[f"ln_b{l}"], np.float32).reshape(1, -1)
    base["scwt"] = np.ascontiguousarray(np.asarray(inputs["score_w"], np.float32).T)
    base["scb"] = np.asarray(inputs["score_b"], np.float32).reshape(1, 1)

    in_maps = []
    for c in range(NCORES):
        m = dict(base)
        m["eidx"] = eidx_all[c]
        m["ind"] = ind_all[c].astype(NP_BF16)
        in_maps.append(m)

    res = run_bass_kernel_spmd(nc, in_maps, list(range(NCORES)))
    kernel.last_results = res

    h = np.concatenate([res.results[c]["h_out"] for c in range(NCORES)], axis=0)
    scores = np.concatenate([res.results[c]["s_out"] for c in range(NCORES)], axis=0)
    return h, scores
